# revision 24
# baseline (speedup 1.0000x reference)
"""TRN2 Bass kernel for nn_Attention_49778670961018 (gnn_message_passing).

Math (per reference):
    S_ss = (Xs @ W_ss.T + b_ss) @ A.T ; S_es = (Xe @ W_es.T + b_es) @ A.T
    w_*  = softmax(S_*, axis=0)   (b_ss/b_es shift each column by a constant
                                   -> no effect on the softmax -> dropped)
    ctx_ss = w_ss.T @ Xs ; ctx_es = w_es.T @ Xe
    out  = tanh([A | ctx_ss | ctx_es] @ W_lin.T + b_lin)

Sharding: attender rows (M=8192) split across 8 cores (1024 each).

Per core (v2 design):
    P0: A^T via PE transpose (fp32r), Q^T = W @ A^T (fp32r matmuls) kept
    resident in fp16; analytic softmax stability bound c[m] = 3.78*||q_m||
    + 40 from ||q|| computed on-chip. X is cast to an fp16 DRAM copy once
    (gpsimd cast-DMA).

    Per set (ss/es) x m-half (4 m-blocks):
      Phase A (scores): X^T arrives by 2-byte DMA xbar transpose straight
      from the fp16 DRAM copy (no PE transposes). S^T[m, n] = Q^T.T @ X^T
      with fp16 operands, fp32 PSUM. exp on ACT with per-partition bias
      -c[m] and Z via accum_out; E^T [m-part, n] stays in SBUF (bf16 -
      fp16 would under/overflow: the margin keeps E ~ e^-40).
      Phase B (agg): E^T is DMA-xbar-transposed SBUF->SBUF into natural
      E [n-part, m] chunks; ctx[m, h] accumulates over ALL n in PSUM
      (4 m-blocks x [128, 1024] fp32 = 8 banks exactly; matmul free dim
      512 = one bank), moving operand X natural fp16 (plain HWDGE read
      on the scalar ring; mixed bf16 x fp16 operands are legal and the
      fp16 X has more mantissa than bf16). 1/Z is a per-partition scale
      at PSUM evacuation. All xbar transposes ride ONE HWDGE ring
      (nc.sync) -- concurrent transposes on both rings corrupt data.

    Final: cat^T built on-chip (A^T cached from P0; ctx^T via PE transpose
    of the small [1024, 1024] ctx blocks); out = tanh(cat^T.T @ W_lin^T + b)
    with W_lin^T from a bf16 DMA transpose. No DRAM round-trips for E or ctx.

Precision: scores fp16 operands (CPU-sim rel err 3.6e-3, same as fp32r);
E bf16, agg X fp16; all accumulation fp32 in PSUM. Measured on HW:
rel err 3.38e-3, 1.198 ms/iter (differential pipelined-dispatch bench).
"""
import os
import sys

import numpy as np

sys.path.insert(0, "/opt/trn_rl_repo")

import concourse.bass as bass  # noqa: E402
import concourse.mybir as mybir  # noqa: E402
import concourse.tile as tile  # noqa: E402
from concourse import bacc  # noqa: E402
from concourse.bass_utils import run_bass_kernel_spmd  # noqa: E402
from concourse.masks import make_identity  # noqa: E402

F32 = mybir.dt.float32
F32R = mybir.dt.float32r
BF16 = mybir.dt.bfloat16
F16 = mybir.dt.float16
AX = mybir.AxisListType
AF = mybir.ActivationFunctionType
ALU = mybir.AluOpType

H = 1024          # hidden dim
HS = H // 128     # h-slices
NCORES = 8
MLOC = 1024       # attender rows per core
MT = MLOC // 128  # m-blocks per core
CMAX_MARGIN = 40.0
XT_AHEAD = 4       # X^T transpose prefetch depth (supers)
EN_AHEAD = 2       # phase-B E/X-nat prefetch depth (supers)


def _max_coef(n):
    """E[max of n iid N(0,1)] (Gumbel asymptotic)."""
    a = np.sqrt(2 * np.log(n))
    return float(a - (np.log(np.log(n)) + np.log(4 * np.pi)) / (2 * a))


def _set_phases(nc, tc, sfx_base, xf16, n_rows, sup, qt16, cneg, ctx_bf,
                xt_ahead=XT_AHEAD, en_ahead=EN_AHEAD):
    """Scores + softmax + aggregation for one attendee set, both m-halves.

    Writes ctx_bf [128, MT, H] (bf16, 1/Z-normalized, m on partitions).
    """
    NSUP = n_rows // sup
    JS = sup // 128
    for half in range(2):
        sfx = f"{sfx_base}h{half}"
        mb0 = half * (MT // 2)
        with (
            tc.tile_pool(name=f"et{sfx}", bufs=1) as etp,
            tc.tile_pool(name=f"z{sfx}", bufs=1) as zp,
            tc.tile_pool(name=f"xt{sfx}", bufs=xt_ahead) as xtp,
            tc.tile_pool(name=f"en{sfx}", bufs=en_ahead + 1) as enp,
            tc.tile_pool(name=f"xn{sfx}", bufs=en_ahead + 1) as xnp,
        ):
            et_all = etp.tile([128, NSUP, 4, sup], BF16,
                              name=f"et{sfx}")
            zc = zp.tile([128, 4, NSUP], F32, name=f"zc{sfx}")
            xts, ens, xns = {}, {}, {}

            def load_xt(isup):
                n0 = isup * sup
                x = xtp.tile([128, HS, sup], F16, tag="xt", name=f"xt{sfx}")
                nc.sync.dma_start_transpose(x[:], xf16[n0:n0 + sup, :])
                xts[isup] = x

            def load_en_xn(isup):
                n0 = isup * sup
                en = enp.tile([128, 4, JS, 128], BF16, tag="en",
                              name=f"en{sfx}")
                nc.sync.dma_start_transpose(en[:], et_all[:, isup, :, :])
                ens[isup] = en
                xn = xnp.tile([128, JS, H], F16, tag="xn", name=f"xn{sfx}")
                nc.scalar.dma_start(
                    xn[:], xf16[n0:n0 + sup, :].rearrange(
                        "(j p) h -> p j h", p=128))
                xns[isup] = xn

            # ---- phase A: scores + exp -> E^T slab (+ first B prefetches)
            with tc.tile_pool(name=f"sps{sfx}", bufs=2, space="PSUM") as sps:
                for isup in range(min(xt_ahead, NSUP)):
                    load_xt(isup)
                for isup in range(NSUP):
                    n0 = isup * sup
                    xt = xts.pop(isup)
                    for i in range(4):
                        mb = mb0 + i
                        sp = sps.tile([128, sup], F32, tag="sp",
                                      name=f"sp{sfx}")
                        hs_abl = 1 if os.environ.get("KABL") == "score1" \
                            else HS
                        for h in range(hs_abl):
                            nc.tensor.matmul(
                                sp[:], qt16[:, h, mb * 128:(mb + 1) * 128],
                                xt[:, h, :], start=(h == 0),
                                stop=(h == hs_abl - 1))
                        nc.scalar.activation(
                            et_all[:, isup, i, :], sp[:], AF.Exp,
                            bias=cneg[:, mb:mb + 1])
                        # Z on the idle DVE instead of ACT's accum_out
                        # (saves ~279 ns of ACT per exp; phase A is
                        # ACT-bound)
                        nc.vector.tensor_reduce(
                            zc[:, i, isup:isup + 1],
                            et_all[:, isup, i, :], axis=AX.X, op=ALU.add)
                    if isup + xt_ahead < NSUP:
                        load_xt(isup + xt_ahead)
                    if isup < en_ahead:
                        # prefetch phase-B inputs so the A->B boundary has
                        # no PE bubble
                        load_en_xn(isup)
            rz = zp.tile([128, 4], F32, name=f"rz{sfx}")
            zt = zp.tile([128, 4], F32, name=f"zt{sfx}")
            for i in range(4):
                nc.vector.tensor_reduce(zt[:, i:i + 1], zc[:, i, :],
                                        axis=AX.X, op=ALU.add)
            nc.vector.reciprocal(rz[:], zt[:])
            # ---- phase B: E^T -> E (SBUF->SBUF xbar) + ctx accumulation
            with tc.tile_pool(name=f"cps{sfx}", bufs=4, space="PSUM") as cps:
                ctx = [cps.tile([128, H], F32, tag="cp", name=f"cp{sfx}")
                       for _ in range(4)]
                for isup in range(NSUP):
                    if isup + en_ahead < NSUP:
                        load_en_xn(isup + en_ahead)
                    en, xn = ens.pop(isup), xns.pop(isup)
                    for j in range(JS):
                        for i in range(4):
                            for hh in range(2):
                                hsl = slice(hh * 512, (hh + 1) * 512)
                                nc.tensor.matmul(
                                    ctx[i][:, hsl], en[:, i, j, :],
                                    xn[:, j, hsl],
                                    start=(isup == 0 and j == 0),
                                    stop=(isup == NSUP - 1 and j == JS - 1))
                for i in range(4):
                    nc.vector.tensor_scalar_mul(
                        ctx_bf[:, mb0 + i, :], ctx[i][:], rz[:, i:i + 1])


def build():
    NS_ROWS = int(os.environ.get("KNS", 8192))
    NE_ROWS = int(os.environ.get("KNE", 4096))
    SUP_S = min(512, NS_ROWS)
    SUP_E = min(512, NE_ROWS)
    nc = bacc.Bacc("TRN2", target_bir_lowering=False, debug=False,
                   num_devices=NCORES)

    xs = nc.dram_tensor("attendee_stmts", [NS_ROWS, H], F32,
                        kind="ExternalInput").ap()
    xe = nc.dram_tensor("attendee_eres", [NE_ROWS, H], F32,
                        kind="ExternalInput").ap()
    al = nc.dram_tensor("attender_loc", [MLOC, H], F32,
                        kind="ExternalInput").ap()
    wss = nc.dram_tensor("W_ss", [H, H], F32, kind="ExternalInput").ap()
    wes = nc.dram_tensor("W_es", [H, H], F32, kind="ExternalInput").ap()
    wlin = nc.dram_tensor("W_lin", [H, 3 * H], F32, kind="ExternalInput").ap()
    blin = nc.dram_tensor("b_lin", [H], F32, kind="ExternalInput").ap()
    out = nc.dram_tensor("out", [MLOC, H], F32, kind="ExternalOutput").ap()

    # DRAM scratch
    xf16_s = nc.dram_tensor("xf16_s", [NS_ROWS, H], F16)
    xf16_e = nc.dram_tensor("xf16_e", [NE_ROWS, H], F16)
    actbf_dram = nc.dram_tensor("actbf_dram", [128, HS, MLOC], BF16)
    cneg_dram = nc.dram_tensor("cneg_dram", [2, MLOC], F32)
    wlin_bf = nc.dram_tensor("wlin_bf", [H, 3 * H], BF16)

    krepeat = int(os.environ.get("KREPEAT", "1"))

    with tile.TileContext(nc) as tc:
      for rep in range(krepeat):
        R = f"r{rep}" if rep else ""
        with (
            tc.tile_pool(name=f"qtes{R}", bufs=1) as qtp_es,
            tc.tile_pool(name=f"small{R}", bufs=1) as small,
        ):
            qt16_es = qtp_es.tile([128, HS, MLOC], F16, name="qt16_es")
            # fp16 X copies (cast-DMA, stripes across all SDMA engines)
            for c4 in range(4):
                r0, r1 = c4 * NS_ROWS // 4, (c4 + 1) * NS_ROWS // 4
                nc.gpsimd.dma_start(xf16_s.ap()[r0:r1], xs[r0:r1])
            for c2 in range(2):
                r0, r1 = c2 * NE_ROWS // 2, (c2 + 1) * NE_ROWS // 2
                nc.gpsimd.dma_start(xf16_e.ap()[r0:r1], xe[r0:r1])
            nc.gpsimd.dma_start(wlin_bf.ap(), wlin)

            ident = small.tile([128, 128], F32)
            make_identity(nc, ident[:])
            ident_bf = small.tile([128, 128], BF16)
            nc.vector.tensor_copy(ident_bf[:], ident[:])
            cneg_ss = small.tile([128, MT], F32)
            cneg_es = small.tile([128, MT], F32)
            ctxbf_ss = small.tile([128, MT, H], BF16, name="ctxbf_ss")
            ctxbf_es = small.tile([128, MT, H], BF16, name="ctxbf_es")

            # ---------------- P0: A^T, Q^T (fp16), c[m] ----------------
            qtp_ss_cm = tc.tile_pool(name=f"qtss{R}", bufs=1)
            qtp_ss = qtp_ss_cm.__enter__()
            qt16_ss = qtp_ss.tile([128, HS, MLOC], F16, name="qt16_ss")
            with (
                tc.tile_pool(name=f"p0{R}", bufs=2) as p0,
                tc.tile_pool(name=f"p0act{R}", bufs=1) as p0act,
                tc.tile_pool(name=f"p0ps{R}", bufs=2, space="PSUM") as p0ps,
                tc.tile_pool(name=f"qnps{R}", bufs=2, space="PSUM") as qnps,
            ):
                # A_c^T via PE transpose (fp32 in, fp32r out via evac)
                act_r = p0act.tile([128, HS, MLOC], F32R, name="act_r")
                for mt in range(MT):
                    a_t = p0.tile([128, H], F32, tag="ald", name="a_t")
                    nc.scalar.dma_start(a_t[:], al[mt * 128:(mt + 1) * 128, :])
                    for hpair in range(HS // 2):
                        pt = p0ps.tile([128, 256], F32, tag="p0t", name="pt0")
                        for i in range(2):
                            h = hpair * 2 + i
                            nc.tensor.transpose(pt[:, i * 128:(i + 1) * 128],
                                                a_t[:, h * 128:(h + 1) * 128],
                                                ident[:])
                        for i in range(2):
                            h = hpair * 2 + i
                            nc.scalar.copy(
                                act_r[:, h, mt * 128:(mt + 1) * 128],
                                pt[:, i * 128:(i + 1) * 128])
                act_bf = p0act.tile([128, HS, MLOC], BF16, name="act_bf")
                nc.vector.tensor_copy(act_bf[:], act_r[:].bitcast(F32))
                nc.gpsimd.dma_start(actbf_dram.ap(), act_bf[:])

                ones_f = p0.tile([128, 1], F32, tag="ones_f", bufs=1,
                                 name="ones_f")
                nc.vector.memset(ones_f[:], 1.0)
                ones_r = p0.tile([128, 1], F32R, tag="ones_r", bufs=1,
                                 name="ones_r")
                nc.vector.tensor_copy(ones_r[:], ones_f[:])

                # Q^T[j, m] = sum_k W[k, j] * A^T[k, m] -> fp16
                for wi, (w_dram, qt16, coef) in enumerate(
                        [(wss, qt16_ss, _max_coef(NS_ROWS)),
                         (wes, qt16_es, _max_coef(NE_ROWS))]):
                    w_r = p0.tile([128, HS, H], F32R, tag="wr", bufs=1,
                                  name="w_r")
                    for k in range(HS):
                        w_t = p0.tile([128, H], F32, tag="wld", name="w_t")
                        nc.scalar.dma_start(w_t[:],
                                            w_dram[k * 128:(k + 1) * 128, :])
                        nc.vector.tensor_copy(w_r[:, k, :], w_t[:])
                    qsq_ps = [qnps.tile([1, 512], F32, tag="qn", name="qn_ps")
                              for _ in range(2)]
                    for j in range(HS):
                        for mh in range(2):
                            qp = p0ps.tile([128, 512], F32, tag="p0q",
                                           name="qp")
                            for k in range(HS):
                                nc.tensor.matmul(
                                    qp[:], w_r[:, k, j * 128:(j + 1) * 128],
                                    act_r[:, k, mh * 512:(mh + 1) * 512],
                                    start=(k == 0), stop=(k == HS - 1))
                            nc.vector.tensor_copy(
                                qt16[:, j, mh * 512:(mh + 1) * 512], qp[:])
                            qsq = p0.tile([128, 512], F32R, tag="qsq",
                                          name="qsq")
                            nc.scalar.activation(qsq[:], qp[:], AF.Square)
                            nc.tensor.matmul(qsq_ps[mh][:], ones_r[:], qsq[:],
                                             start=(j == 0),
                                             stop=(j == HS - 1))
                    qn_row = p0.tile([1, MLOC], F32, tag="qn_row",
                                     name="qn_row")
                    for mh in range(2):
                        nc.scalar.activation(qn_row[:, mh * 512:(mh + 1) * 512],
                                             qsq_ps[mh][:], AF.Sqrt)
                    cn_row = p0.tile([1, MLOC], F32, tag="cn_row",
                                     name="cn_row")
                    nc.vector.tensor_scalar(cn_row[:], qn_row[:], -coef,
                                            -CMAX_MARGIN, op0=ALU.mult,
                                            op1=ALU.add)
                    nc.sync.dma_start(cneg_dram.ap()[wi, :], cn_row[0:1, :])
                nc.sync.dma_start(
                    cneg_ss[:],
                    cneg_dram.ap()[0, :].rearrange("(m p) -> p m", p=128))
                nc.sync.dma_start(
                    cneg_es[:],
                    cneg_dram.ap()[1, :].rearrange("(m p) -> p m", p=128))

            # ---------------- ss set: scores + agg ----------------
            _set_phases(nc, tc, f"s{R}", xf16_s.ap(), NS_ROWS, SUP_S,
                        qt16_ss, cneg_ss, ctxbf_ss)
            qtp_ss_cm.__exit__(None, None, None)

            if os.environ.get("KDBG") and rep == 0:
                dbg_qt = nc.dram_tensor("dbg_qt", [128, HS, MLOC], F16,
                                        kind="ExternalOutput")
                nc.sync.dma_start(dbg_qt.ap(), qt16_ss[:])
                dbg_cn = nc.dram_tensor("dbg_cn", [128, MT], F32,
                                        kind="ExternalOutput")
                nc.sync.dma_start(dbg_cn.ap(), cneg_ss[:])
                dbg_ctx = nc.dram_tensor("dbg_ctx", [128, MT, H], BF16,
                                         kind="ExternalOutput")
                nc.sync.dma_start(dbg_ctx.ap(), ctxbf_ss[:])

            # -------- es set (final-linear weights prefetch first) --------
            with tc.tile_pool(name=f"p45{R}", bufs=1) as p45:
                wlt = p45.tile([128, 3 * HS, H], BF16, name="wlt")
                nc.sync.dma_start_transpose(wlt[:], wlin_bf.ap())
                blin_bf = p45.tile([1, H], BF16, name="blin_bf")
                nc.gpsimd.dma_start(blin_bf[:],
                                    blin.rearrange("(a h) -> a h", a=1))
                ones_bf = p45.tile([1, 128], BF16, name="ones_bf")
                nc.vector.memset(ones_bf[:], 1.0)

                _set_phases(nc, tc, f"e{R}", xf16_e.ap(), NE_ROWS, SUP_E,
                            qt16_es, cneg_es, ctxbf_es,
                            xt_ahead=2, en_ahead=1)
                acbf = p45.tile([128, HS, MLOC], BF16, name="acbf")
                nc.gpsimd.dma_start(acbf[:], actbf_dram.ap())

                # ---------- ctx^T via PE transpose ----------
                with (
                    tc.tile_pool(name=f"ctT{R}", bufs=1) as ctTp,
                    tc.tile_pool(name=f"ctp{R}", bufs=2, space="PSUM") as ctp,
                ):
                    ctxT_ss = ctTp.tile([128, HS, MLOC], BF16, name="ctxT_ss")
                    ctxT_es = ctTp.tile([128, HS, MLOC], BF16, name="ctxT_es")
                    for cbf, ctT in ((ctxbf_ss, ctxT_ss),
                                     (ctxbf_es, ctxT_es)):
                        for mt in range(MT):
                            # full-bank PSUM tile (2 KiB): PE-W and DVE-R of
                            # rotating bufs never share a bank
                            pt = ctp.tile([128, HS, 128], BF16, tag="pt",
                                          name=f"ptc{R}")
                            for h in range(HS):
                                nc.tensor.transpose(
                                    pt[:, h, :],
                                    cbf[:, mt, h * 128:(h + 1) * 128],
                                    ident_bf[:])
                            nc.vector.tensor_copy(
                                ctT[:, :, mt * 128:(mt + 1) * 128], pt[:])

                    if os.environ.get("KDBG") and rep == 0:
                        dbg_ce = nc.dram_tensor("dbg_ce", [128, MT, H], BF16,
                                                kind="ExternalOutput")
                        nc.sync.dma_start(dbg_ce.ap(), ctxbf_es[:])
                        dbg_ct = nc.dram_tensor("dbg_ct", [128, HS, MLOC],
                                                BF16, kind="ExternalOutput")
                        nc.sync.dma_start(dbg_ct.ap(), ctxT_ss[:])
                        dbg_ac = nc.dram_tensor("dbg_ac", [128, HS, MLOC],
                                                BF16, kind="ExternalOutput")
                        nc.sync.dma_start(dbg_ac.ap(), acbf[:])
                        dbg_wl = nc.dram_tensor("dbg_wl", [128, 3 * HS, H],
                                                BF16, kind="ExternalOutput")
                        nc.sync.dma_start(dbg_wl.ap(), wlt[:])

                    # ---------- final linear + tanh ----------
                    with (
                        tc.tile_pool(name=f"p5o{R}", bufs=4) as p5o,
                        tc.tile_pool(name=f"p5ps{R}", bufs=4,
                                     space="PSUM") as p5ps,
                    ):
                        for m in range(MT):
                            msl = slice(m * 128, (m + 1) * 128)
                            for ah in range(2):
                                fp = p5ps.tile([128, 512], F32, tag="fp",
                                               name="fp")
                                asl = slice(ah * 512, (ah + 1) * 512)
                                for s in range(HS):
                                    nc.tensor.matmul(fp[:], acbf[:, s, msl],
                                                     wlt[:, s, asl],
                                                     start=(s == 0),
                                                     stop=False)
                                for s in range(HS):
                                    nc.tensor.matmul(fp[:], ctxT_ss[:, s, msl],
                                                     wlt[:, HS + s, asl],
                                                     start=False, stop=False)
                                for s in range(HS):
                                    nc.tensor.matmul(fp[:], ctxT_es[:, s, msl],
                                                     wlt[:, 2 * HS + s, asl],
                                                     start=False, stop=False)
                                nc.tensor.matmul(fp[:], ones_bf[0:1, :],
                                                 blin_bf[0:1, asl],
                                                 start=False, stop=True)
                                o_sb = p5o.tile([128, 512], F32, tag="o_sb",
                                                name="o_sb")
                                nc.scalar.activation(o_sb[:], fp[:], AF.Tanh)
                                nc.sync.dma_start(out[msl, asl], o_sb[:])

    nc.compile()
    return nc


_NC_CACHE = None


def kernel(**inputs):
    global _NC_CACHE
    xs = np.ascontiguousarray(np.asarray(inputs["attendee_stmts"],
                                         dtype=np.float32))
    xe = np.ascontiguousarray(np.asarray(inputs["attendee_eres"],
                                         dtype=np.float32))
    att = np.ascontiguousarray(np.asarray(inputs["attender"],
                                          dtype=np.float32))
    wss = np.ascontiguousarray(np.asarray(inputs["W_ss"], dtype=np.float32))
    wes = np.ascontiguousarray(np.asarray(inputs["W_es"], dtype=np.float32))
    wlin = np.ascontiguousarray(np.asarray(inputs["W_lin"], dtype=np.float32))
    blin = np.ascontiguousarray(np.asarray(inputs["b_lin"], dtype=np.float32))

    if _NC_CACHE is None:
        _NC_CACHE = build()
    nc = _NC_CACHE

    in_maps = []
    for c in range(NCORES):
        in_maps.append({
            "attendee_stmts": xs,
            "attendee_eres": xe,
            "attender_loc": np.ascontiguousarray(att[c * MLOC:(c + 1) * MLOC, :]),
            "W_ss": wss,
            "W_es": wes,
            "W_lin": wlin,
            "b_lin": blin,
        })
    trace = bool(int(os.environ.get("KTRACE", "0")))
    res = run_bass_kernel_spmd(nc, in_maps, core_ids=list(range(NCORES)),
                               trace=trace)
    global LAST_RESULTS
    LAST_RESULTS = res
    return np.concatenate(
        [res.results[c]["out"] for c in range(NCORES)], axis=0).astype(np.float32)


LAST_RESULTS = None


# revision 27
# speedup vs baseline: 1.0404x; 1.0404x over previous
"""TRN2 Bass kernel for nn_Attention_49778670961018 (gnn_message_passing).

Math (per reference):
    S_ss = (Xs @ W_ss.T + b_ss) @ A.T ; S_es = (Xe @ W_es.T + b_es) @ A.T
    w_*  = softmax(S_*, axis=0)   (b_ss/b_es shift each column by a constant
                                   -> no effect on the softmax -> dropped)
    ctx_ss = w_ss.T @ Xs ; ctx_es = w_es.T @ Xe
    out  = tanh([A | ctx_ss | ctx_es] @ W_lin.T + b_lin)

Sharding: attender rows (M=8192) split across 8 cores (1024 each).

Per core (v2 design):
    P0: A^T via PE transpose (fp32r), Q^T = W @ A^T (fp32r matmuls) kept
    resident in fp16; analytic softmax stability bound c[m] = 3.78*||q_m||
    + 40 from ||q|| computed on-chip. X is cast to an fp16 DRAM copy once
    (gpsimd cast-DMA).

    Per set (ss/es) x m-half (4 m-blocks):
      Phase A (scores): X^T arrives by 2-byte DMA xbar transpose straight
      from the fp16 DRAM copy (no PE transposes). S^T[m, n] = Q^T.T @ X^T
      with fp16 operands, fp32 PSUM. exp on ACT with per-partition bias
      -c[m] and Z via accum_out; E^T [m-part, n] stays in SBUF (bf16 -
      fp16 would under/overflow: the margin keeps E ~ e^-40).
      Phase B (agg): E^T is DMA-xbar-transposed SBUF->SBUF into natural
      E [n-part, m] chunks; ctx[m, h] accumulates over ALL n in PSUM
      (4 m-blocks x [128, 1024] fp32 = 8 banks exactly; matmul free dim
      512 = one bank), moving operand X natural fp16 (plain HWDGE read
      on the scalar ring; mixed bf16 x fp16 operands are legal and the
      fp16 X has more mantissa than bf16). 1/Z is a per-partition scale
      at PSUM evacuation. All xbar transposes ride ONE HWDGE ring
      (nc.sync) -- concurrent transposes on both rings corrupt data.

    Final: cat^T built on-chip (A^T cached from P0; ctx^T via PE transpose
    of the small [1024, 1024] ctx blocks); out = tanh(cat^T.T @ W_lin^T + b)
    with W_lin^T from a bf16 DMA transpose. No DRAM round-trips for E or ctx.

Precision: scores fp16 operands (CPU-sim rel err 3.6e-3, same as fp32r);
E bf16, agg X fp16; all accumulation fp32 in PSUM. Measured on HW:
rel err 3.38e-3, 1.198 ms/iter (differential pipelined-dispatch bench).
"""
import os
import sys

import numpy as np

sys.path.insert(0, "/opt/trn_rl_repo")

import concourse.bass as bass  # noqa: E402
import concourse.mybir as mybir  # noqa: E402
import concourse.tile as tile  # noqa: E402
from concourse import bacc  # noqa: E402
from concourse.bass_utils import run_bass_kernel_spmd  # noqa: E402
from concourse.masks import make_identity  # noqa: E402

F32 = mybir.dt.float32
F32R = mybir.dt.float32r
BF16 = mybir.dt.bfloat16
F16 = mybir.dt.float16
AX = mybir.AxisListType
AF = mybir.ActivationFunctionType
ALU = mybir.AluOpType

H = 1024          # hidden dim
HS = H // 128     # h-slices
NCORES = 8
MLOC = 1024       # attender rows per core
MT = MLOC // 128  # m-blocks per core
CMAX_MARGIN = 40.0
XT_AHEAD = 4       # X^T transpose prefetch depth (supers)
EN_AHEAD = 2       # phase-B E/X-nat prefetch depth (supers)


def _max_coef(n):
    """E[max of n iid N(0,1)] (Gumbel asymptotic)."""
    a = np.sqrt(2 * np.log(n))
    return float(a - (np.log(np.log(n)) + np.log(4 * np.pi)) / (2 * a))


def _set_phases(nc, tc, sfx_base, xf16, n_rows, sup, qt16, cneg, ctx_bf,
                xt_ahead=XT_AHEAD, en_ahead=EN_AHEAD):
    """Scores + softmax + aggregation for one attendee set, both m-halves.

    Writes ctx_bf [128, MT, H] (bf16, 1/Z-normalized, m on partitions).
    """
    NSUP = n_rows // sup
    JS = sup // 128
    for half in range(2):
        sfx = f"{sfx_base}h{half}"
        mb0 = half * (MT // 2)
        with (
            tc.tile_pool(name=f"et{sfx}", bufs=1) as etp,
            tc.tile_pool(name=f"z{sfx}", bufs=1) as zp,
            tc.tile_pool(name=f"xt{sfx}", bufs=xt_ahead) as xtp,
            tc.tile_pool(name=f"en{sfx}", bufs=en_ahead + 1) as enp,
            tc.tile_pool(name=f"xn{sfx}", bufs=en_ahead + 1) as xnp,
        ):
            et_all = etp.tile([128, NSUP, 4, sup], BF16,
                              name=f"et{sfx}")
            zc = zp.tile([128, 4, NSUP], F32, name=f"zc{sfx}")
            xts, ens, xns = {}, {}, {}

            def load_xt(isup):
                n0 = isup * sup
                x = xtp.tile([128, HS, sup], F16, tag="xt", name=f"xt{sfx}")
                nc.sync.dma_start_transpose(x[:], xf16[n0:n0 + sup, :])
                xts[isup] = x

            def load_en_xn(isup):
                n0 = isup * sup
                en = enp.tile([128, 4, JS, 128], BF16, tag="en",
                              name=f"en{sfx}")
                nc.sync.dma_start_transpose(en[:], et_all[:, isup, :, :])
                ens[isup] = en
                xn = xnp.tile([128, JS, H], F16, tag="xn", name=f"xn{sfx}")
                nc.scalar.dma_start(
                    xn[:], xf16[n0:n0 + sup, :].rearrange(
                        "(j p) h -> p j h", p=128))
                xns[isup] = xn

            # ---- phase A: scores + exp -> E^T slab (+ first B prefetches)
            with tc.tile_pool(name=f"sps{sfx}", bufs=2, space="PSUM") as sps:
                for isup in range(min(xt_ahead, NSUP)):
                    load_xt(isup)
                for isup in range(NSUP):
                    n0 = isup * sup
                    xt = xts.pop(isup)
                    for i in range(4):
                        mb = mb0 + i
                        sp = sps.tile([128, sup], F32, tag="sp",
                                      name=f"sp{sfx}")
                        for h in range(HS):
                            nc.tensor.matmul(
                                sp[:], qt16[:, h, mb * 128:(mb + 1) * 128],
                                xt[:, h, :], start=(h == 0),
                                stop=(h == HS - 1))
                        nc.scalar.activation(
                            et_all[:, isup, i, :], sp[:], AF.Exp,
                            bias=cneg[:, mb:mb + 1],
                            accum_out=zc[:, i, isup:isup + 1])
                    if isup + xt_ahead < NSUP:
                        load_xt(isup + xt_ahead)
                    if isup < en_ahead:
                        # prefetch phase-B inputs so the A->B boundary has
                        # no PE bubble
                        load_en_xn(isup)
            rz = zp.tile([128, 4], F32, name=f"rz{sfx}")
            zt = zp.tile([128, 4], F32, name=f"zt{sfx}")
            for i in range(4):
                nc.vector.tensor_reduce(zt[:, i:i + 1], zc[:, i, :],
                                        axis=AX.X, op=ALU.add)
            nc.vector.reciprocal(rz[:], zt[:])
            # ---- phase B: E^T -> E (SBUF->SBUF xbar) + ctx accumulation
            with tc.tile_pool(name=f"cps{sfx}", bufs=4, space="PSUM") as cps:
                ctx = [cps.tile([128, H], F32, tag="cp", name=f"cp{sfx}")
                       for _ in range(4)]
                for isup in range(NSUP):
                    if isup + en_ahead < NSUP:
                        load_en_xn(isup + en_ahead)
                    en, xn = ens.pop(isup), xns.pop(isup)
                    for j in range(JS):
                        for i in range(4):
                            for hh in range(2):
                                hsl = slice(hh * 512, (hh + 1) * 512)
                                nc.tensor.matmul(
                                    ctx[i][:, hsl], en[:, i, j, :],
                                    xn[:, j, hsl],
                                    start=(isup == 0 and j == 0),
                                    stop=(isup == NSUP - 1 and j == JS - 1))
                for i in range(4):
                    nc.vector.tensor_scalar_mul(
                        ctx_bf[:, mb0 + i, :], ctx[i][:], rz[:, i:i + 1])


def build():
    NS_ROWS = int(os.environ.get("KNS", 8192))
    NE_ROWS = int(os.environ.get("KNE", 4096))
    SUP_S = min(512, NS_ROWS)
    SUP_E = min(512, NE_ROWS)
    nc = bacc.Bacc("TRN2", target_bir_lowering=False, debug=False,
                   num_devices=NCORES)

    xs = nc.dram_tensor("attendee_stmts", [NS_ROWS, H], F32,
                        kind="ExternalInput").ap()
    xe = nc.dram_tensor("attendee_eres", [NE_ROWS, H], F32,
                        kind="ExternalInput").ap()
    al = nc.dram_tensor("attender_loc", [MLOC, H], F32,
                        kind="ExternalInput").ap()
    wss = nc.dram_tensor("W_ss", [H, H], F32, kind="ExternalInput").ap()
    wes = nc.dram_tensor("W_es", [H, H], F32, kind="ExternalInput").ap()
    wlin = nc.dram_tensor("W_lin", [H, 3 * H], F32, kind="ExternalInput").ap()
    blin = nc.dram_tensor("b_lin", [H], F32, kind="ExternalInput").ap()
    out = nc.dram_tensor("out", [MLOC, H], F32, kind="ExternalOutput").ap()

    # DRAM scratch
    xf16_s = nc.dram_tensor("xf16_s", [NS_ROWS, H], F16)
    xf16_e = nc.dram_tensor("xf16_e", [NE_ROWS, H], F16)
    actbf_dram = nc.dram_tensor("actbf_dram", [128, HS, MLOC], BF16)
    cneg_dram = nc.dram_tensor("cneg_dram", [2, MLOC], F32)
    wlin_bf = nc.dram_tensor("wlin_bf", [H, 3 * H], BF16)

    krepeat = int(os.environ.get("KREPEAT", "1"))

    with tile.TileContext(nc) as tc:
      for rep in range(krepeat):
        R = f"r{rep}" if rep else ""
        with (
            tc.tile_pool(name=f"qtes{R}", bufs=1) as qtp_es,
            tc.tile_pool(name=f"small{R}", bufs=1) as small,
        ):
            qt16_es = qtp_es.tile([128, HS, MLOC], F16, name="qt16_es")
            # fp16 X copies (cast-DMA, stripes across all SDMA engines)
            for c4 in range(4):
                r0, r1 = c4 * NS_ROWS // 4, (c4 + 1) * NS_ROWS // 4
                nc.gpsimd.dma_start(xf16_s.ap()[r0:r1], xs[r0:r1])
            for c2 in range(2):
                r0, r1 = c2 * NE_ROWS // 2, (c2 + 1) * NE_ROWS // 2
                nc.gpsimd.dma_start(xf16_e.ap()[r0:r1], xe[r0:r1])
            nc.gpsimd.dma_start(wlin_bf.ap(), wlin)

            ident = small.tile([128, 128], F32)
            make_identity(nc, ident[:])
            ident_bf = small.tile([128, 128], BF16)
            nc.vector.tensor_copy(ident_bf[:], ident[:])
            cneg_ss = small.tile([128, MT], F32)
            cneg_es = small.tile([128, MT], F32)
            ctxbf_ss = small.tile([128, MT, H], BF16, name="ctxbf_ss")
            ctxbf_es = small.tile([128, MT, H], BF16, name="ctxbf_es")

            # ---------------- P0: A^T, Q^T (fp16), c[m] ----------------
            qtp_ss_cm = tc.tile_pool(name=f"qtss{R}", bufs=1)
            qtp_ss = qtp_ss_cm.__enter__()
            qt16_ss = qtp_ss.tile([128, HS, MLOC], F16, name="qt16_ss")
            with (
                tc.tile_pool(name=f"p0{R}", bufs=2) as p0,
                tc.tile_pool(name=f"p0act{R}", bufs=1) as p0act,
                tc.tile_pool(name=f"p0ps{R}", bufs=2, space="PSUM") as p0ps,
                tc.tile_pool(name=f"qnps{R}", bufs=2, space="PSUM") as qnps,
            ):
                # A_c^T via PE transpose (fp32 in, fp32r out via evac)
                act_r = p0act.tile([128, HS, MLOC], F32R, name="act_r")
                for mt in range(MT):
                    a_t = p0.tile([128, H], F32, tag="ald", name="a_t")
                    nc.scalar.dma_start(a_t[:], al[mt * 128:(mt + 1) * 128, :])
                    for hpair in range(HS // 2):
                        pt = p0ps.tile([128, 256], F32, tag="p0t", name="pt0")
                        for i in range(2):
                            h = hpair * 2 + i
                            nc.tensor.transpose(pt[:, i * 128:(i + 1) * 128],
                                                a_t[:, h * 128:(h + 1) * 128],
                                                ident[:])
                        for i in range(2):
                            h = hpair * 2 + i
                            nc.scalar.copy(
                                act_r[:, h, mt * 128:(mt + 1) * 128],
                                pt[:, i * 128:(i + 1) * 128])
                act_bf = p0act.tile([128, HS, MLOC], BF16, name="act_bf")
                nc.vector.tensor_copy(act_bf[:], act_r[:].bitcast(F32))
                nc.gpsimd.dma_start(actbf_dram.ap(), act_bf[:])

                ones_f = p0.tile([128, 1], F32, tag="ones_f", bufs=1,
                                 name="ones_f")
                nc.vector.memset(ones_f[:], 1.0)
                ones_r = p0.tile([128, 1], F32R, tag="ones_r", bufs=1,
                                 name="ones_r")
                nc.vector.tensor_copy(ones_r[:], ones_f[:])

                # Q^T[j, m] = sum_k W[k, j] * A^T[k, m] -> fp16
                for wi, (w_dram, qt16, coef) in enumerate(
                        [(wss, qt16_ss, _max_coef(NS_ROWS)),
                         (wes, qt16_es, _max_coef(NE_ROWS))]):
                    w_r = p0.tile([128, HS, H], F32R, tag="wr", bufs=1,
                                  name="w_r")
                    for k in range(HS):
                        w_t = p0.tile([128, H], F32, tag="wld", name="w_t")
                        nc.scalar.dma_start(w_t[:],
                                            w_dram[k * 128:(k + 1) * 128, :])
                        nc.vector.tensor_copy(w_r[:, k, :], w_t[:])
                    qsq_ps = [qnps.tile([1, 512], F32, tag="qn", name="qn_ps")
                              for _ in range(2)]
                    for j in range(HS):
                        for mh in range(2):
                            qp = p0ps.tile([128, 512], F32, tag="p0q",
                                           name="qp")
                            for k in range(HS):
                                nc.tensor.matmul(
                                    qp[:], w_r[:, k, j * 128:(j + 1) * 128],
                                    act_r[:, k, mh * 512:(mh + 1) * 512],
                                    start=(k == 0), stop=(k == HS - 1))
                            nc.vector.tensor_copy(
                                qt16[:, j, mh * 512:(mh + 1) * 512], qp[:])
                            qsq = p0.tile([128, 512], F32R, tag="qsq",
                                          name="qsq")
                            nc.scalar.activation(qsq[:], qp[:], AF.Square)
                            nc.tensor.matmul(qsq_ps[mh][:], ones_r[:], qsq[:],
                                             start=(j == 0),
                                             stop=(j == HS - 1))
                    qn_row = p0.tile([1, MLOC], F32, tag="qn_row",
                                     name="qn_row")
                    for mh in range(2):
                        nc.scalar.activation(qn_row[:, mh * 512:(mh + 1) * 512],
                                             qsq_ps[mh][:], AF.Sqrt)
                    cn_row = p0.tile([1, MLOC], F32, tag="cn_row",
                                     name="cn_row")
                    nc.vector.tensor_scalar(cn_row[:], qn_row[:], -coef,
                                            -CMAX_MARGIN, op0=ALU.mult,
                                            op1=ALU.add)
                    nc.sync.dma_start(cneg_dram.ap()[wi, :], cn_row[0:1, :])
                nc.sync.dma_start(
                    cneg_ss[:],
                    cneg_dram.ap()[0, :].rearrange("(m p) -> p m", p=128))
                nc.sync.dma_start(
                    cneg_es[:],
                    cneg_dram.ap()[1, :].rearrange("(m p) -> p m", p=128))

            # ---------------- ss set: scores + agg ----------------
            _set_phases(nc, tc, f"s{R}", xf16_s.ap(), NS_ROWS, SUP_S,
                        qt16_ss, cneg_ss, ctxbf_ss)
            qtp_ss_cm.__exit__(None, None, None)

            if os.environ.get("KDBG") and rep == 0:
                dbg_qt = nc.dram_tensor("dbg_qt", [128, HS, MLOC], F16,
                                        kind="ExternalOutput")
                nc.sync.dma_start(dbg_qt.ap(), qt16_ss[:])
                dbg_cn = nc.dram_tensor("dbg_cn", [128, MT], F32,
                                        kind="ExternalOutput")
                nc.sync.dma_start(dbg_cn.ap(), cneg_ss[:])
                dbg_ctx = nc.dram_tensor("dbg_ctx", [128, MT, H], BF16,
                                         kind="ExternalOutput")
                nc.sync.dma_start(dbg_ctx.ap(), ctxbf_ss[:])

            # -------- es set (final-linear weights prefetch first) --------
            with tc.tile_pool(name=f"p45{R}", bufs=1) as p45:
                wlt = p45.tile([128, 3 * HS, H], BF16, name="wlt")
                nc.sync.dma_start_transpose(wlt[:], wlin_bf.ap())
                blin_bf = p45.tile([1, H], BF16, name="blin_bf")
                nc.gpsimd.dma_start(blin_bf[:],
                                    blin.rearrange("(a h) -> a h", a=1))
                ones_bf = p45.tile([1, 128], BF16, name="ones_bf")
                nc.vector.memset(ones_bf[:], 1.0)

                _set_phases(nc, tc, f"e{R}", xf16_e.ap(), NE_ROWS, SUP_E,
                            qt16_es, cneg_es, ctxbf_es,
                            xt_ahead=2, en_ahead=1)
                acbf = p45.tile([128, HS, MLOC], BF16, name="acbf")
                nc.gpsimd.dma_start(acbf[:], actbf_dram.ap())

                # ---------- ctx^T via PE transpose ----------
                with (
                    tc.tile_pool(name=f"ctT{R}", bufs=1) as ctTp,
                    tc.tile_pool(name=f"ctp{R}", bufs=2, space="PSUM") as ctp,
                ):
                    ctxT_ss = ctTp.tile([128, HS, MLOC], BF16, name="ctxT_ss")
                    ctxT_es = ctTp.tile([128, HS, MLOC], BF16, name="ctxT_es")
                    for cbf, ctT in ((ctxbf_ss, ctxT_ss),
                                     (ctxbf_es, ctxT_es)):
                        for mt in range(MT):
                            # full-bank PSUM tile (2 KiB): PE-W and DVE-R of
                            # rotating bufs never share a bank
                            pt = ctp.tile([128, HS, 128], BF16, tag="pt",
                                          name=f"ptc{R}")
                            for h in range(HS):
                                nc.tensor.transpose(
                                    pt[:, h, :],
                                    cbf[:, mt, h * 128:(h + 1) * 128],
                                    ident_bf[:])
                            nc.vector.tensor_copy(
                                ctT[:, :, mt * 128:(mt + 1) * 128], pt[:])

                    if os.environ.get("KDBG") and rep == 0:
                        dbg_ce = nc.dram_tensor("dbg_ce", [128, MT, H], BF16,
                                                kind="ExternalOutput")
                        nc.sync.dma_start(dbg_ce.ap(), ctxbf_es[:])
                        dbg_ct = nc.dram_tensor("dbg_ct", [128, HS, MLOC],
                                                BF16, kind="ExternalOutput")
                        nc.sync.dma_start(dbg_ct.ap(), ctxT_ss[:])
                        dbg_ac = nc.dram_tensor("dbg_ac", [128, HS, MLOC],
                                                BF16, kind="ExternalOutput")
                        nc.sync.dma_start(dbg_ac.ap(), acbf[:])
                        dbg_wl = nc.dram_tensor("dbg_wl", [128, 3 * HS, H],
                                                BF16, kind="ExternalOutput")
                        nc.sync.dma_start(dbg_wl.ap(), wlt[:])

                    # ---------- final linear + tanh ----------
                    with (
                        tc.tile_pool(name=f"p5o{R}", bufs=4) as p5o,
                        tc.tile_pool(name=f"p5ps{R}", bufs=4,
                                     space="PSUM") as p5ps,
                    ):
                        for m in range(MT):
                            msl = slice(m * 128, (m + 1) * 128)
                            for ah in range(2):
                                fp = p5ps.tile([128, 512], F32, tag="fp",
                                               name="fp")
                                asl = slice(ah * 512, (ah + 1) * 512)
                                for s in range(HS):
                                    nc.tensor.matmul(fp[:], acbf[:, s, msl],
                                                     wlt[:, s, asl],
                                                     start=(s == 0),
                                                     stop=False)
                                for s in range(HS):
                                    nc.tensor.matmul(fp[:], ctxT_ss[:, s, msl],
                                                     wlt[:, HS + s, asl],
                                                     start=False, stop=False)
                                for s in range(HS):
                                    nc.tensor.matmul(fp[:], ctxT_es[:, s, msl],
                                                     wlt[:, 2 * HS + s, asl],
                                                     start=False, stop=False)
                                nc.tensor.matmul(fp[:], ones_bf[0:1, :],
                                                 blin_bf[0:1, asl],
                                                 start=False, stop=True)
                                o_sb = p5o.tile([128, 512], F32, tag="o_sb",
                                                name="o_sb")
                                nc.scalar.activation(o_sb[:], fp[:], AF.Tanh)
                                nc.sync.dma_start(out[msl, asl], o_sb[:])

    nc.compile()
    return nc


_NC_CACHE = None


def kernel(**inputs):
    global _NC_CACHE
    xs = np.ascontiguousarray(np.asarray(inputs["attendee_stmts"],
                                         dtype=np.float32))
    xe = np.ascontiguousarray(np.asarray(inputs["attendee_eres"],
                                         dtype=np.float32))
    att = np.ascontiguousarray(np.asarray(inputs["attender"],
                                          dtype=np.float32))
    wss = np.ascontiguousarray(np.asarray(inputs["W_ss"], dtype=np.float32))
    wes = np.ascontiguousarray(np.asarray(inputs["W_es"], dtype=np.float32))
    wlin = np.ascontiguousarray(np.asarray(inputs["W_lin"], dtype=np.float32))
    blin = np.ascontiguousarray(np.asarray(inputs["b_lin"], dtype=np.float32))

    if _NC_CACHE is None:
        _NC_CACHE = build()
    nc = _NC_CACHE

    in_maps = []
    for c in range(NCORES):
        in_maps.append({
            "attendee_stmts": xs,
            "attendee_eres": xe,
            "attender_loc": np.ascontiguousarray(att[c * MLOC:(c + 1) * MLOC, :]),
            "W_ss": wss,
            "W_es": wes,
            "W_lin": wlin,
            "b_lin": blin,
        })
    trace = bool(int(os.environ.get("KTRACE", "0")))
    res = run_bass_kernel_spmd(nc, in_maps, core_ids=list(range(NCORES)),
                               trace=trace)
    global LAST_RESULTS
    LAST_RESULTS = res
    return np.concatenate(
        [res.results[c]["out"] for c in range(NCORES)], axis=0).astype(np.float32)


LAST_RESULTS = None
